# revision 24
# baseline (speedup 1.0000x reference)
"""AddShift_mp_linear_module on 8 TRN2 NeuronCores.

Strategy (channel-block sharding, no collectives):
  - 96 output-channel blocks (11 input channels each) -> 12 blocks/core.
  - Every branch is a contraction over the block's (k, spatial) axis:
      out_v[co, h, (b,w)]  = sum_{k,h'} Ov[(k,h'), h]   * x[b, c, h', w]
      out_i[co, h, (b,w)]  = sum_{k,h'} Oi[(k,h'), h]   * x[b, c, h', w]
      out_h[co, w, (b,h')] = sum_{k,w'} Oh[(k,w'), w]   * x[b, c, h', w']
    with sparse operators Ov/Oi/Oh built on the host from
    w1/w2/w3/pad_hv/idx_identit (all known at call time).
  - K = 660 rows = 5 chunks of 128 (+ 20-row tail packed separately:
    HWDGE splits descriptors by partition group, so only 128-partition
    transfers engage all 16 SDMA engines).
  - Per block ONE fused DRAM tensor packs [x_v | x_h | opv | oph] as
    [128 partitions, 5 chunks, 1080 cols] -> a single 1.38MB DMA with
    10800B-contiguous per-partition runs (opv padded to 128 cols for
    FWL). Tails for all blocks ride one resident [20, 12, 1080] tensor
    on the gpsimd SWDGE queue (software DGE round-robins bytes across
    all 16 engines regardless of partition count).
  - Engine roles kept disjoint to avoid in-order sequencer jams:
      sync/scalar -> 6 block DMAs each (both HWDGE rings; per-ring DMAs
                     serialize, so both rings are needed for HBM rate)
      tensor -> dense warmup burst (un-throttles the HAM clock gate off
                a memset tile, not gated on any DMA) + matmuls + dummy
                matmuls after each block (idle windows re-throttle)
      scalar -> V-branch PSUM->SBUF copies (after its DMA issues)
      vector -> memset + I/H-branch copies
      gpsimd -> tails DMA + 12 output DMAs
    All x DMAs are issued up-front into 12 resident tiles (no
    buffer-reuse waits), so the rings never starve.
  - Outputs leave as [56, 3, 448] bf16 tiles (one DMA per block); host
    restores (out_h, out_v, out_i) [b, co, h, w] fp32.
"""

import numpy as np
import ml_dtypes

# architecture constants (match reference init_kwargs)
B = 8
C_OUT = 96
NK = 11
G = 4
C_IN = C_OUT * NK          # 1056
HOUT = WOUT = 56
HIN = WIN = 60
EP = 2                     # extra pad
N_CORES = 8
BPC = C_OUT // N_CORES     # blocks per core = 12
CPC = BPC * NK             # channels per core = 132
KROWS = NK * HIN           # 660 contraction rows per block
KCH = 128                  # K-chunk size
NCH = 5                    # full chunks per block
KTAIL = KROWS - NCH * KCH  # 20 tail rows
NFREE = B * WOUT           # 448 matmul free dim (w/h pre-sliced to [2,58))
MV = 128                   # opv cols: V at 0:56, identity at 64:120, pad to 128 (FWL)
MH = 56                    # oph cols
TW = 2 * NFREE + MV + MH   # 1072 packed free width
NWARM = 24                 # dense warmup burst (un-throttles HAM)
NDUMMY = 2                 # keep-warm dummies appended after each block
SYNC_BLOCKS = (0, 2, 4, 6, 8, 10)      # ring balance: qAct starts ~3us late

BF16 = ml_dtypes.bfloat16

_CACHE = {}


def _build_operators(w1, w2, w3, pad_hv, idx_identit):
    """Build per-block stationary operators.

    Returns opv (96, 660, 120) fp32  [cols 0:56 = V, 64:120 = identity]
            oph (96, 660, 56)  fp32
    Row r = k*60 + spatial_in, for channel c = co*11 + k.
    """
    w1r = np.asarray(w1, np.float32).reshape(G, C_IN)
    w2r = np.asarray(w2, np.float32).reshape(G, C_IN)
    w3r = np.asarray(w3, np.float32).reshape(G, C_OUT)
    pad = np.asarray(pad_hv, np.int64)            # (C_IN, 2G)
    idx = np.asarray(idx_identit, np.int64)       # (C_OUT, G)

    opv = np.zeros((C_OUT, KROWS, MV), np.float32)  # cols 120:128 stay zero
    oph = np.zeros((C_OUT, KROWS, MH), np.float32)

    c_all = np.arange(C_IN)
    co_all = c_all // NK
    k_all = c_all % NK
    pos = np.arange(HOUT)                          # output spatial index

    for g in range(G):
        # horizontal: w_in = w_out + EP + pad[c, g]
        win = pos[None, :] + EP + pad[:, g][:, None]        # (C_IN, 56)
        ok = (win >= 0) & (win < WIN)
        cc, oo = np.nonzero(ok)
        np.add.at(oph, (co_all[cc], k_all[cc] * HIN + win[cc, oo], oo), w1r[g, cc])
        # vertical: h_in = h_out + EP + pad[c, G+g]
        hin = pos[None, :] + EP + pad[:, G + g][:, None]
        ok = (hin >= 0) & (hin < HIN)
        cc, oo = np.nonzero(ok)
        np.add.at(opv, (co_all[cc], k_all[cc] * HIN + hin[cc, oo], oo), w2r[g, cc])

    # identity: out_i[co] = sum_g w3r[g, co] * x[idx[co, g]] (idx within block co)
    k_sel = idx - np.arange(C_OUT)[:, None] * NK            # (C_OUT, G)
    assert np.all((k_sel >= 0) & (k_sel < NK)), "idx_identit outside its block"
    u = np.zeros((C_OUT, NK), np.float32)
    for g in range(G):
        np.add.at(u, (np.arange(C_OUT), k_sel[:, g]), w3r[g])
    co_i, k_i = np.nonzero(u != 0)
    for co, k in zip(co_i, k_i):
        opv[co, k * HIN + pos + EP, 64 + pos] += u[co, k]
    return opv, oph


def _build_nc():
    import concourse.bacc as bacc
    import concourse.tile as tile
    import concourse.bass as bass
    import concourse.mybir as mybir
    from contextlib import ExitStack

    f32 = mybir.dt.float32
    bf16 = mybir.dt.bfloat16

    nc = bacc.Bacc(None, target_bir_lowering=False)
    xall_d = nc.declare_dram_parameter(
        "xall", [BPC, KCH, NCH, TW], bf16, isOutput=False)
    xt_d = nc.declare_dram_parameter(
        "xtail", [KTAIL, BPC, TW], bf16, isOutput=False)
    out_d = nc.declare_dram_parameter("out", [BPC, 56, 3, NFREE], bf16, isOutput=True)

    with tile.TileContext(nc) as tc, ExitStack() as ctx:
        x_pool = ctx.enter_context(tc.tile_pool(name="x", bufs=BPC))
        w_pool = ctx.enter_context(tc.tile_pool(name="warm", bufs=1))
        o_pool = ctx.enter_context(tc.tile_pool(name="outs", bufs=3))
        psum_pool = ctx.enter_context(
            tc.tile_pool(name="psum", bufs=3, space=bass.MemorySpace.PSUM)
        )
        psum_wp = ctx.enter_context(
            tc.tile_pool(name="psumw", bufs=1, space=bass.MemorySpace.PSUM)
        )

        # tails for all blocks: one resident SWDGE DMA, issued first
        tails = w_pool.tile([KTAIL, BPC, TW], bf16, tag="tails")
        nc.gpsimd.dma_start(tails[:], xt_d[:])

        # one full-block DMA per block, alternating the two HWDGE rings;
        # all issued up-front into 12 resident tiles
        tiles = []
        for bi in range(BPC):
            t = x_pool.tile([KCH, NCH, TW], bf16, tag="xt")
            if bi == BPC - 1:
                # split the last block across both rings: it lands ~2us
                # earlier and with less arrival jitter (a >3.4us PE idle
                # right before the final block re-throttles the clock)
                nc.sync.dma_start(t[:, 0:3, :], xall_d[bi, :, 0:3, :])
                nc.scalar.dma_start(t[:, 3:5, :], xall_d[bi, :, 3:5, :])
            else:
                (nc.sync if bi in SYNC_BLOCKS else nc.scalar).dma_start(
                    t[:], xall_d[bi])
            tiles.append(t)

        # PE warmup off a memset tile: a dense burst right after the
        # preamble un-throttles the HAM clock gate before real data lands
        wt = w_pool.tile([KCH, NFREE], bf16, tag="warm")
        nc.vector.memset(wt[:], 0.0)
        pw = psum_wp.tile([128, NFREE], f32, tag="pw")
        for _ in range(NWARM):
            nc.tensor.matmul(pw[:], wt[:, :128], wt[:], start=True, stop=True)

        for bi in range(BPC):
            t = tiles[bi]
            psum_vi = psum_pool.tile([MV, NFREE], f32, tag="pv")
            psum_h = psum_pool.tile([MH, NFREE], f32, tag="ph")
            # interleave the two accumulation chains so PE drains overlap;
            # the V chain finishes (stop) two matmuls early so its ACT-copy
            # drain overlaps the H chain's last matmuls
            for j in range(NCH):
                nc.tensor.matmul(
                    psum_vi[:], t[:, j, 2 * NFREE:2 * NFREE + MV], t[:, j, :NFREE],
                    start=(j == 0), stop=False,
                )
                if j < NCH - 1:
                    nc.tensor.matmul(
                        psum_h[:], t[:, j, 2 * NFREE + MV:], t[:, j, NFREE:2 * NFREE],
                        start=(j == 0), stop=False,
                    )
            nc.tensor.matmul(
                psum_vi[:], tails[:, bi, 2 * NFREE:2 * NFREE + MV],
                tails[:, bi, :NFREE], start=False, stop=True,
            )
            nc.tensor.matmul(
                psum_h[:], t[:, NCH - 1, 2 * NFREE + MV:],
                t[:, NCH - 1, NFREE:2 * NFREE], start=False, stop=False,
            )
            nc.tensor.matmul(
                psum_h[:], tails[:, bi, 2 * NFREE + MV:],
                tails[:, bi, NFREE:2 * NFREE], start=False, stop=True,
            )
            # stage [56, (3, 448)] bf16: slot 0 = V, 1 = I, 2 = H; one DMA out
            st = o_pool.tile([56, 3, NFREE], bf16, tag="st")
            nc.scalar.copy(st[:, 0, :], psum_vi[:56])
            nc.vector.tensor_copy(st[:, 1, :], psum_vi[64:120])
            nc.vector.tensor_copy(st[:, 2, :], psum_h[:])
            # last block's store goes on the (by-then idle) scalar HWDGE
            # ring: ~1us lower first-byte latency than SWDGE shortens the tail
            (nc.scalar if bi == BPC - 1 else nc.gpsimd).dma_start(out_d[bi], st[:])
            if bi < BPC - 1:
                # a short same-stationary burst after each block keeps the
                # HAM activity monitor reading "busy" (real matmuls reload
                # the stationary every time, which HAM seems to discount);
                # kept small so the PE never becomes the critical path.
                # Extra cushion near the tail, where arrival jitter is worst.
                # Half-width (N=224) dummies: same burst density for the
                # activity monitor at half the PE cost.
                for _ in range(NDUMMY + (6 if bi >= 7 else 0)):
                    nc.tensor.matmul(pw[:, :224], wt[:, :128], wt[:, :224],
                                     start=True, stop=True)
    nc.finalize()
    return nc


def prepare_inputs(x, w1, w2, w3, pad_hv, idx_identit):
    """Host-side shard prep. Returns in_maps (list of 8 dicts)."""
    x = np.asarray(x)
    xb = x.astype(BF16)                                   # (B, C, 60, 60)
    # h-major for V/I: [c, h', (b, w in [2,58))]
    x_hbw = np.ascontiguousarray(
        xb[:, :, :, EP:EP + WOUT].transpose(1, 2, 0, 3)).reshape(C_IN * HIN, NFREE)
    # w-major for H: [c, w', (b, h in [2,58))]
    x_wbh = np.ascontiguousarray(
        xb[:, :, EP:EP + HOUT, :].transpose(1, 3, 0, 2)).reshape(C_IN * WIN, NFREE)

    opv, oph = _build_operators(w1, w2, w3, pad_hv, idx_identit)
    opv16 = opv.astype(BF16)                               # (96, 660, 120)
    oph16 = oph.astype(BF16)                               # (96, 660, 56)

    nm = NCH * KCH                                         # 640 main rows

    def chunked(a):  # (BPC, 640, M) -> (BPC, KCH, NCH, M)
        m = a.shape[2]
        return a.reshape(BPC, NCH, KCH, m).transpose(0, 2, 1, 3)

    in_maps = []
    for i in range(N_CORES):
        r0 = i * CPC * HIN
        xv_r = x_hbw[r0:r0 + CPC * HIN].reshape(BPC, KROWS, NFREE)
        xh_r = x_wbh[r0:r0 + CPC * WIN].reshape(BPC, KROWS, NFREE)
        ov = opv16[i * BPC:(i + 1) * BPC]                  # (BPC, 660, 120)
        oh = oph16[i * BPC:(i + 1) * BPC]
        xall = np.concatenate(
            [chunked(xv_r[:, :nm]), chunked(xh_r[:, :nm]),
             chunked(ov[:, :nm]), chunked(oh[:, :nm])],
            axis=3)                                        # (BPC, 128, 5, 1072)
        xtail = np.concatenate(
            [xv_r[:, nm:], xh_r[:, nm:], ov[:, nm:], oh[:, nm:]],
            axis=2)                                        # (BPC, 20, 1072)
        in_maps.append({
            "xall": np.ascontiguousarray(xall),
            "xtail": np.ascontiguousarray(xtail.transpose(1, 0, 2)),
        })
    return in_maps


def unshard(results):
    """results: list of 8 dicts with 'out' (BPC, 56, 3, 448) bf16 ->
    (out_h, out_v, out_i) each (B, C_OUT, 56, 56) fp32."""
    O = np.stack([np.asarray(r["out"], np.float32) for r in results])  # (8,12,56,3,448)
    O = O.reshape(N_CORES, BPC, 56, 3, B, WOUT)
    # (core, co_l, h, b, w) -> (b, core, co_l, h, w)
    out_v = O[:, :, :, 0].transpose(3, 0, 1, 2, 4).reshape(B, C_OUT, HOUT, WOUT)
    out_i = O[:, :, :, 1].transpose(3, 0, 1, 2, 4).reshape(B, C_OUT, HOUT, WOUT)
    h = O[:, :, :, 2]                          # (core, co_l, w, b, h)
    out_h = h.transpose(3, 0, 1, 4, 2).reshape(B, C_OUT, HOUT, WOUT)
    return out_h, out_v, out_i


def kernel(x, w1, w2, w3, pad_hv, idx_identit, b=B, hout=HOUT, wout=WOUT):
    from concourse.bass_utils import run_bass_kernel_spmd

    assert int(b) == B and int(hout) == HOUT and int(wout) == WOUT
    assert tuple(np.asarray(x).shape) == (B, C_IN, HIN, WIN)

    in_maps = prepare_inputs(x, w1, w2, w3, pad_hv, idx_identit)
    nc = _CACHE.get("nc")
    if nc is None:
        nc = _build_nc()
        _CACHE["nc"] = nc
    res = run_bass_kernel_spmd(nc, in_maps, core_ids=list(range(N_CORES)))
    return unshard(res.results)


# revision 26
# speedup vs baseline: 1.2034x; 1.2034x over previous
"""AddShift_mp_linear_module on 8 TRN2 NeuronCores.

Strategy (channel-block sharding, no collectives):
  - 96 output-channel blocks (11 input channels each) -> 12 blocks/core.
  - Every branch is a contraction over the block's (k, spatial) axis:
      out_v[co, h, (b,w)]  = sum_{k,h'} Ov[(k,h'), h]   * x[b, c, h', w]
      out_i[co, h, (b,w)]  = sum_{k,h'} Oi[(k,h'), h]   * x[b, c, h', w]
      out_h[co, w, (b,h')] = sum_{k,w'} Oh[(k,w'), w]   * x[b, c, h', w']
    with sparse operators Ov/Oi/Oh built on the host from
    w1/w2/w3/pad_hv/idx_identit (all known at call time).
  - K = 660 rows = 5 chunks of 128 (+ 20-row tail packed separately:
    HWDGE splits descriptors by partition group, so only 128-partition
    transfers engage all 16 SDMA engines).
  - Per block ONE fused DRAM tensor packs [x_v | x_h | opv | oph] as
    [128 partitions, 5 chunks, 1080 cols] -> a single 1.38MB DMA with
    10800B-contiguous per-partition runs (opv padded to 128 cols for
    FWL). Tails for all blocks ride one resident [20, 12, 1080] tensor
    on the gpsimd SWDGE queue (software DGE round-robins bytes across
    all 16 engines regardless of partition count).
  - Engine roles kept disjoint to avoid in-order sequencer jams:
      sync/scalar -> 6 block DMAs each (both HWDGE rings; per-ring DMAs
                     serialize, so both rings are needed for HBM rate)
      tensor -> dense warmup burst (un-throttles the HAM clock gate off
                a memset tile, not gated on any DMA) + matmuls + dummy
                matmuls after each block (idle windows re-throttle)
      scalar -> V-branch PSUM->SBUF copies (after its DMA issues)
      vector -> memset + I/H-branch copies
      gpsimd -> tails DMA + 12 output DMAs
    All x DMAs are issued up-front into 12 resident tiles (no
    buffer-reuse waits), so the rings never starve.
  - Outputs leave as [56, 3, 448] bf16 tiles (one DMA per block); host
    restores (out_h, out_v, out_i) [b, co, h, w] fp32.
"""

import numpy as np
import ml_dtypes

# architecture constants (match reference init_kwargs)
B = 8
C_OUT = 96
NK = 11
G = 4
C_IN = C_OUT * NK          # 1056
HOUT = WOUT = 56
HIN = WIN = 60
EP = 2                     # extra pad
N_CORES = 8
BPC = C_OUT // N_CORES     # blocks per core = 12
CPC = BPC * NK             # channels per core = 132
KROWS = NK * HIN           # 660 contraction rows per block
KCH = 128                  # K-chunk size
NCH = 5                    # full chunks per block
KTAIL = KROWS - NCH * KCH  # 20 tail rows
NFREE = B * WOUT           # 448 matmul free dim (w/h pre-sliced to [2,58))
MV = 128                   # opv cols: V at 0:56, identity at 64:120, pad to 128 (FWL)
MH = 56                    # oph cols
TW = 2 * NFREE + MV + MH   # 1072 packed free width
NWARM = 40                 # dense warmup burst (un-throttles HAM); sized to
                           # keep the PE busy until block 0 lands (~19us) so
                           # the clock gate never re-throttles over blocks 0-1
NDUMMY = 2                 # keep-warm dummies appended after each block
SYNC_BLOCKS = (0, 2, 4, 6, 8, 10)      # ring balance: qAct starts ~3us late

BF16 = ml_dtypes.bfloat16

_CACHE = {}


def _build_operators(w1, w2, w3, pad_hv, idx_identit):
    """Build per-block stationary operators.

    Returns opv (96, 660, 120) fp32  [cols 0:56 = V, 64:120 = identity]
            oph (96, 660, 56)  fp32
    Row r = k*60 + spatial_in, for channel c = co*11 + k.
    """
    w1r = np.asarray(w1, np.float32).reshape(G, C_IN)
    w2r = np.asarray(w2, np.float32).reshape(G, C_IN)
    w3r = np.asarray(w3, np.float32).reshape(G, C_OUT)
    pad = np.asarray(pad_hv, np.int64)            # (C_IN, 2G)
    idx = np.asarray(idx_identit, np.int64)       # (C_OUT, G)

    opv = np.zeros((C_OUT, KROWS, MV), np.float32)  # cols 120:128 stay zero
    oph = np.zeros((C_OUT, KROWS, MH), np.float32)

    c_all = np.arange(C_IN)
    co_all = c_all // NK
    k_all = c_all % NK
    pos = np.arange(HOUT)                          # output spatial index

    for g in range(G):
        # horizontal: w_in = w_out + EP + pad[c, g]
        win = pos[None, :] + EP + pad[:, g][:, None]        # (C_IN, 56)
        ok = (win >= 0) & (win < WIN)
        cc, oo = np.nonzero(ok)
        np.add.at(oph, (co_all[cc], k_all[cc] * HIN + win[cc, oo], oo), w1r[g, cc])
        # vertical: h_in = h_out + EP + pad[c, G+g]
        hin = pos[None, :] + EP + pad[:, G + g][:, None]
        ok = (hin >= 0) & (hin < HIN)
        cc, oo = np.nonzero(ok)
        np.add.at(opv, (co_all[cc], k_all[cc] * HIN + hin[cc, oo], oo), w2r[g, cc])

    # identity: out_i[co] = sum_g w3r[g, co] * x[idx[co, g]] (idx within block co)
    k_sel = idx - np.arange(C_OUT)[:, None] * NK            # (C_OUT, G)
    assert np.all((k_sel >= 0) & (k_sel < NK)), "idx_identit outside its block"
    u = np.zeros((C_OUT, NK), np.float32)
    for g in range(G):
        np.add.at(u, (np.arange(C_OUT), k_sel[:, g]), w3r[g])
    co_i, k_i = np.nonzero(u != 0)
    for co, k in zip(co_i, k_i):
        opv[co, k * HIN + pos + EP, 64 + pos] += u[co, k]
    return opv, oph


def _build_nc():
    import concourse.bacc as bacc
    import concourse.tile as tile
    import concourse.bass as bass
    import concourse.mybir as mybir
    from contextlib import ExitStack

    f32 = mybir.dt.float32
    bf16 = mybir.dt.bfloat16

    nc = bacc.Bacc(None, target_bir_lowering=False)
    xall_d = nc.declare_dram_parameter(
        "xall", [BPC, KCH, NCH, TW], bf16, isOutput=False)
    xt_d = nc.declare_dram_parameter(
        "xtail", [KTAIL, BPC, TW], bf16, isOutput=False)
    out_d = nc.declare_dram_parameter("out", [BPC, 56, 3, NFREE], bf16, isOutput=True)

    with tile.TileContext(nc) as tc, ExitStack() as ctx:
        x_pool = ctx.enter_context(tc.tile_pool(name="x", bufs=BPC))
        w_pool = ctx.enter_context(tc.tile_pool(name="warm", bufs=1))
        o_pool = ctx.enter_context(tc.tile_pool(name="outs", bufs=3))
        psum_pool = ctx.enter_context(
            tc.tile_pool(name="psum", bufs=3, space=bass.MemorySpace.PSUM)
        )
        psum_wp = ctx.enter_context(
            tc.tile_pool(name="psumw", bufs=1, space=bass.MemorySpace.PSUM)
        )

        # tails for all blocks: one resident SWDGE DMA, issued first
        tails = w_pool.tile([KTAIL, BPC, TW], bf16, tag="tails")
        nc.gpsimd.dma_start(tails[:], xt_d[:])

        # one full-block DMA per block, alternating the two HWDGE rings;
        # all issued up-front into 12 resident tiles
        tiles = []
        for bi in range(BPC):
            t = x_pool.tile([KCH, NCH, TW], bf16, tag="xt")
            if bi == BPC - 1:
                # split the last block across both rings: it lands ~2us
                # earlier and with less arrival jitter (a >3.4us PE idle
                # right before the final block re-throttles the clock)
                nc.sync.dma_start(t[:, 0:3, :], xall_d[bi, :, 0:3, :])
                nc.scalar.dma_start(t[:, 3:5, :], xall_d[bi, :, 3:5, :])
            else:
                (nc.sync if bi in SYNC_BLOCKS else nc.scalar).dma_start(
                    t[:], xall_d[bi])
            tiles.append(t)

        # PE warmup off a memset tile: a dense burst right after the
        # preamble un-throttles the HAM clock gate before real data lands
        wt = w_pool.tile([KCH, NFREE], bf16, tag="warm")
        nc.vector.memset(wt[:], 0.0)
        pw = psum_wp.tile([128, NFREE], f32, tag="pw")
        for _ in range(NWARM):
            nc.tensor.matmul(pw[:], wt[:, :128], wt[:], start=True, stop=True)

        for bi in range(BPC):
            t = tiles[bi]
            psum_vi = psum_pool.tile([MV, NFREE], f32, tag="pv")
            psum_h = psum_pool.tile([MH, NFREE], f32, tag="ph")
            # interleave the two accumulation chains so PE drains overlap;
            # the V chain finishes (stop) two matmuls early so its ACT-copy
            # drain overlaps the H chain's last matmuls
            for j in range(NCH):
                nc.tensor.matmul(
                    psum_vi[:], t[:, j, 2 * NFREE:2 * NFREE + MV], t[:, j, :NFREE],
                    start=(j == 0), stop=False,
                )
                if j < NCH - 1:
                    nc.tensor.matmul(
                        psum_h[:], t[:, j, 2 * NFREE + MV:], t[:, j, NFREE:2 * NFREE],
                        start=(j == 0), stop=False,
                    )
            nc.tensor.matmul(
                psum_vi[:], tails[:, bi, 2 * NFREE:2 * NFREE + MV],
                tails[:, bi, :NFREE], start=False, stop=True,
            )
            nc.tensor.matmul(
                psum_h[:], t[:, NCH - 1, 2 * NFREE + MV:],
                t[:, NCH - 1, NFREE:2 * NFREE], start=False, stop=False,
            )
            nc.tensor.matmul(
                psum_h[:], tails[:, bi, 2 * NFREE + MV:],
                tails[:, bi, NFREE:2 * NFREE], start=False, stop=True,
            )
            # stage [56, (3, 448)] bf16: slot 0 = V, 1 = I, 2 = H; one DMA out
            st = o_pool.tile([56, 3, NFREE], bf16, tag="st")
            nc.scalar.copy(st[:, 0, :], psum_vi[:56])
            nc.vector.tensor_copy(st[:, 1, :], psum_vi[64:120])
            nc.vector.tensor_copy(st[:, 2, :], psum_h[:])
            # last block's store goes on the (by-then idle) scalar HWDGE
            # ring: ~1us lower first-byte latency than SWDGE shortens the tail
            (nc.scalar if bi == BPC - 1 else nc.gpsimd).dma_start(out_d[bi], st[:])
            if bi < BPC - 1:
                # a short same-stationary burst after each block keeps the
                # HAM activity monitor reading "busy" (real matmuls reload
                # the stationary every time, which HAM seems to discount);
                # kept small so the PE never becomes the critical path.
                # Extra cushion near the tail, where arrival jitter is worst.
                # Full-width (N=448) dummies are required: half-width ones
                # measured 92us — the activity monitor counts busy wall-time
                # coverage, so cheaper bursts let the clock re-throttle.
                for _ in range(NDUMMY + (6 if bi >= 7 else 0)):
                    nc.tensor.matmul(pw[:], wt[:, :128], wt[:],
                                     start=True, stop=True)
    nc.finalize()
    return nc


def prepare_inputs(x, w1, w2, w3, pad_hv, idx_identit):
    """Host-side shard prep. Returns in_maps (list of 8 dicts)."""
    x = np.asarray(x)
    xb = x.astype(BF16)                                   # (B, C, 60, 60)
    # h-major for V/I: [c, h', (b, w in [2,58))]
    x_hbw = np.ascontiguousarray(
        xb[:, :, :, EP:EP + WOUT].transpose(1, 2, 0, 3)).reshape(C_IN * HIN, NFREE)
    # w-major for H: [c, w', (b, h in [2,58))]
    x_wbh = np.ascontiguousarray(
        xb[:, :, EP:EP + HOUT, :].transpose(1, 3, 0, 2)).reshape(C_IN * WIN, NFREE)

    opv, oph = _build_operators(w1, w2, w3, pad_hv, idx_identit)
    opv16 = opv.astype(BF16)                               # (96, 660, 120)
    oph16 = oph.astype(BF16)                               # (96, 660, 56)

    nm = NCH * KCH                                         # 640 main rows

    def chunked(a):  # (BPC, 640, M) -> (BPC, KCH, NCH, M)
        m = a.shape[2]
        return a.reshape(BPC, NCH, KCH, m).transpose(0, 2, 1, 3)

    in_maps = []
    for i in range(N_CORES):
        r0 = i * CPC * HIN
        xv_r = x_hbw[r0:r0 + CPC * HIN].reshape(BPC, KROWS, NFREE)
        xh_r = x_wbh[r0:r0 + CPC * WIN].reshape(BPC, KROWS, NFREE)
        ov = opv16[i * BPC:(i + 1) * BPC]                  # (BPC, 660, 120)
        oh = oph16[i * BPC:(i + 1) * BPC]
        xall = np.concatenate(
            [chunked(xv_r[:, :nm]), chunked(xh_r[:, :nm]),
             chunked(ov[:, :nm]), chunked(oh[:, :nm])],
            axis=3)                                        # (BPC, 128, 5, 1072)
        xtail = np.concatenate(
            [xv_r[:, nm:], xh_r[:, nm:], ov[:, nm:], oh[:, nm:]],
            axis=2)                                        # (BPC, 20, 1072)
        in_maps.append({
            "xall": np.ascontiguousarray(xall),
            "xtail": np.ascontiguousarray(xtail.transpose(1, 0, 2)),
        })
    return in_maps


def unshard(results):
    """results: list of 8 dicts with 'out' (BPC, 56, 3, 448) bf16 ->
    (out_h, out_v, out_i) each (B, C_OUT, 56, 56) fp32."""
    O = np.stack([np.asarray(r["out"], np.float32) for r in results])  # (8,12,56,3,448)
    O = O.reshape(N_CORES, BPC, 56, 3, B, WOUT)
    # (core, co_l, h, b, w) -> (b, core, co_l, h, w)
    out_v = O[:, :, :, 0].transpose(3, 0, 1, 2, 4).reshape(B, C_OUT, HOUT, WOUT)
    out_i = O[:, :, :, 1].transpose(3, 0, 1, 2, 4).reshape(B, C_OUT, HOUT, WOUT)
    h = O[:, :, :, 2]                          # (core, co_l, w, b, h)
    out_h = h.transpose(3, 0, 1, 4, 2).reshape(B, C_OUT, HOUT, WOUT)
    return out_h, out_v, out_i


def kernel(x, w1, w2, w3, pad_hv, idx_identit, b=B, hout=HOUT, wout=WOUT):
    from concourse.bass_utils import run_bass_kernel_spmd

    assert int(b) == B and int(hout) == HOUT and int(wout) == WOUT
    assert tuple(np.asarray(x).shape) == (B, C_IN, HIN, WIN)

    in_maps = prepare_inputs(x, w1, w2, w3, pad_hv, idx_identit)
    nc = _CACHE.get("nc")
    if nc is None:
        nc = _build_nc()
        _CACHE["nc"] = nc
    res = run_bass_kernel_spmd(nc, in_maps, core_ids=list(range(N_CORES)))
    return unshard(res.results)


# revision 27
# speedup vs baseline: 1.2088x; 1.0045x over previous
"""AddShift_mp_linear_module on 8 TRN2 NeuronCores.

Strategy (channel-block sharding, no collectives):
  - 96 output-channel blocks (11 input channels each) -> 12 blocks/core.
  - Every branch is a contraction over the block's (k, spatial) axis:
      out_v[co, h, (b,w)]  = sum_{k,h'} Ov[(k,h'), h]   * x[b, c, h', w]
      out_i[co, h, (b,w)]  = sum_{k,h'} Oi[(k,h'), h]   * x[b, c, h', w]
      out_h[co, w, (b,h')] = sum_{k,w'} Oh[(k,w'), w]   * x[b, c, h', w']
    with sparse operators Ov/Oi/Oh built on the host from
    w1/w2/w3/pad_hv/idx_identit (all known at call time).
  - K = 660 rows = 5 chunks of 128 (+ 20-row tail packed separately:
    HWDGE splits descriptors by partition group, so only 128-partition
    transfers engage all 16 SDMA engines).
  - Per block ONE fused DRAM tensor packs [x_v | x_h | opv | oph] as
    [128 partitions, 5 chunks, 1080 cols] -> a single 1.38MB DMA with
    10800B-contiguous per-partition runs (opv padded to 128 cols for
    FWL). Tails for all blocks ride one resident [20, 12, 1080] tensor
    on the gpsimd SWDGE queue (software DGE round-robins bytes across
    all 16 engines regardless of partition count).
  - Engine roles kept disjoint to avoid in-order sequencer jams:
      sync/scalar -> 6 block DMAs each (both HWDGE rings; per-ring DMAs
                     serialize, so both rings are needed for HBM rate)
      tensor -> dense warmup burst (un-throttles the HAM clock gate off
                a memset tile, not gated on any DMA) + matmuls + dummy
                matmuls after each block (idle windows re-throttle)
      scalar -> V-branch PSUM->SBUF copies (after its DMA issues)
      vector -> memset + I/H-branch copies
      gpsimd -> tails DMA + 12 output DMAs
    All x DMAs are issued up-front into 12 resident tiles (no
    buffer-reuse waits), so the rings never starve.
  - Outputs leave as [56, 3, 448] bf16 tiles (one DMA per block); host
    restores (out_h, out_v, out_i) [b, co, h, w] fp32.
"""

import numpy as np
import ml_dtypes

# architecture constants (match reference init_kwargs)
B = 8
C_OUT = 96
NK = 11
G = 4
C_IN = C_OUT * NK          # 1056
HOUT = WOUT = 56
HIN = WIN = 60
EP = 2                     # extra pad
N_CORES = 8
BPC = C_OUT // N_CORES     # blocks per core = 12
CPC = BPC * NK             # channels per core = 132
KROWS = NK * HIN           # 660 contraction rows per block
KCH = 128                  # K-chunk size
NCH = 5                    # full chunks per block
KTAIL = KROWS - NCH * KCH  # 20 tail rows
NFREE = B * WOUT           # 448 matmul free dim (w/h pre-sliced to [2,58))
MV = 128                   # opv cols: V at 0:56, identity at 64:120, pad to 128 (FWL)
MH = 56                    # oph cols
TW = 2 * NFREE + MV + MH   # 1072 packed free width
NWARM = 40                 # dense warmup burst (un-throttles HAM); sized to
                           # keep the PE busy until block 0 lands (~19us) so
                           # the clock gate never re-throttles over blocks 0-1
NDUMMY = 2                 # keep-warm dummies appended after each block
SYNC_BLOCKS = (0, 2, 4, 6, 8, 10)      # ring balance: qAct starts ~3us late

BF16 = ml_dtypes.bfloat16

_CACHE = {}


def _build_operators(w1, w2, w3, pad_hv, idx_identit):
    """Build per-block stationary operators.

    Returns opv (96, 660, 120) fp32  [cols 0:56 = V, 64:120 = identity]
            oph (96, 660, 56)  fp32
    Row r = k*60 + spatial_in, for channel c = co*11 + k.
    """
    w1r = np.asarray(w1, np.float32).reshape(G, C_IN)
    w2r = np.asarray(w2, np.float32).reshape(G, C_IN)
    w3r = np.asarray(w3, np.float32).reshape(G, C_OUT)
    pad = np.asarray(pad_hv, np.int64)            # (C_IN, 2G)
    idx = np.asarray(idx_identit, np.int64)       # (C_OUT, G)

    opv = np.zeros((C_OUT, KROWS, MV), np.float32)  # cols 120:128 stay zero
    oph = np.zeros((C_OUT, KROWS, MH), np.float32)

    c_all = np.arange(C_IN)
    co_all = c_all // NK
    k_all = c_all % NK
    pos = np.arange(HOUT)                          # output spatial index

    for g in range(G):
        # horizontal: w_in = w_out + EP + pad[c, g]
        win = pos[None, :] + EP + pad[:, g][:, None]        # (C_IN, 56)
        ok = (win >= 0) & (win < WIN)
        cc, oo = np.nonzero(ok)
        np.add.at(oph, (co_all[cc], k_all[cc] * HIN + win[cc, oo], oo), w1r[g, cc])
        # vertical: h_in = h_out + EP + pad[c, G+g]
        hin = pos[None, :] + EP + pad[:, G + g][:, None]
        ok = (hin >= 0) & (hin < HIN)
        cc, oo = np.nonzero(ok)
        np.add.at(opv, (co_all[cc], k_all[cc] * HIN + hin[cc, oo], oo), w2r[g, cc])

    # identity: out_i[co] = sum_g w3r[g, co] * x[idx[co, g]] (idx within block co)
    k_sel = idx - np.arange(C_OUT)[:, None] * NK            # (C_OUT, G)
    assert np.all((k_sel >= 0) & (k_sel < NK)), "idx_identit outside its block"
    u = np.zeros((C_OUT, NK), np.float32)
    for g in range(G):
        np.add.at(u, (np.arange(C_OUT), k_sel[:, g]), w3r[g])
    co_i, k_i = np.nonzero(u != 0)
    for co, k in zip(co_i, k_i):
        opv[co, k * HIN + pos + EP, 64 + pos] += u[co, k]
    return opv, oph


def _build_nc():
    import concourse.bacc as bacc
    import concourse.tile as tile
    import concourse.bass as bass
    import concourse.mybir as mybir
    from contextlib import ExitStack

    f32 = mybir.dt.float32
    bf16 = mybir.dt.bfloat16

    nc = bacc.Bacc(None, target_bir_lowering=False)
    xall_d = nc.declare_dram_parameter(
        "xall", [BPC, KCH, NCH, TW], bf16, isOutput=False)
    xt_d = nc.declare_dram_parameter(
        "xtail", [KTAIL, BPC, TW], bf16, isOutput=False)
    out_d = nc.declare_dram_parameter("out", [BPC, 56, 3, NFREE], bf16, isOutput=True)

    with tile.TileContext(nc) as tc, ExitStack() as ctx:
        x_pool = ctx.enter_context(tc.tile_pool(name="x", bufs=BPC))
        w_pool = ctx.enter_context(tc.tile_pool(name="warm", bufs=1))
        o_pool = ctx.enter_context(tc.tile_pool(name="outs", bufs=3))
        psum_pool = ctx.enter_context(
            tc.tile_pool(name="psum", bufs=3, space=bass.MemorySpace.PSUM)
        )
        psum_wp = ctx.enter_context(
            tc.tile_pool(name="psumw", bufs=1, space=bass.MemorySpace.PSUM)
        )

        # tails for all blocks: one resident SWDGE DMA, issued first
        tails = w_pool.tile([KTAIL, BPC, TW], bf16, tag="tails")
        nc.gpsimd.dma_start(tails[:], xt_d[:])

        # one full-block DMA per block, alternating the two HWDGE rings;
        # all issued up-front into 12 resident tiles
        tiles = []
        for bi in range(BPC):
            t = x_pool.tile([KCH, NCH, TW], bf16, tag="xt")
            if bi == BPC - 1:
                # split the last block across both rings: it lands ~2us
                # earlier and with less arrival jitter (a >3.4us PE idle
                # right before the final block re-throttles the clock)
                nc.sync.dma_start(t[:, 0:3, :], xall_d[bi, :, 0:3, :])
                nc.scalar.dma_start(t[:, 3:5, :], xall_d[bi, :, 3:5, :])
            else:
                (nc.sync if bi in SYNC_BLOCKS else nc.scalar).dma_start(
                    t[:], xall_d[bi])
            tiles.append(t)

        # PE warmup off a memset tile: a dense burst right after the
        # preamble un-throttles the HAM clock gate before real data lands
        wt = w_pool.tile([KCH, NFREE], bf16, tag="warm")
        nc.vector.memset(wt[:], 0.0)
        pw = psum_wp.tile([128, NFREE], f32, tag="pw")
        for _ in range(NWARM):
            nc.tensor.matmul(pw[:], wt[:, :128], wt[:], start=True, stop=True)

        for bi in range(BPC):
            t = tiles[bi]
            psum_vi = psum_pool.tile([MV, NFREE], f32, tag="pv")
            psum_h = psum_pool.tile([MH, NFREE], f32, tag="ph")
            # interleave the two accumulation chains so PE drains overlap;
            # the V chain finishes (stop) two matmuls early so its ACT-copy
            # drain overlaps the H chain's last matmuls
            for j in range(NCH):
                nc.tensor.matmul(
                    psum_vi[:], t[:, j, 2 * NFREE:2 * NFREE + MV], t[:, j, :NFREE],
                    start=(j == 0), stop=False,
                )
                if j < NCH - 1:
                    nc.tensor.matmul(
                        psum_h[:], t[:, j, 2 * NFREE + MV:], t[:, j, NFREE:2 * NFREE],
                        start=(j == 0), stop=False,
                    )
            nc.tensor.matmul(
                psum_vi[:], tails[:, bi, 2 * NFREE:2 * NFREE + MV],
                tails[:, bi, :NFREE], start=False, stop=True,
            )
            nc.tensor.matmul(
                psum_h[:], t[:, NCH - 1, 2 * NFREE + MV:],
                t[:, NCH - 1, NFREE:2 * NFREE], start=False, stop=False,
            )
            nc.tensor.matmul(
                psum_h[:], tails[:, bi, 2 * NFREE + MV:],
                tails[:, bi, NFREE:2 * NFREE], start=False, stop=True,
            )
            # stage [56, (3, 448)] bf16: slot 0 = V, 1 = I, 2 = H; one DMA out
            st = o_pool.tile([56, 3, NFREE], bf16, tag="st")
            nc.scalar.copy(st[:, 0, :], psum_vi[:56])
            nc.vector.tensor_copy(st[:, 1, :], psum_vi[64:120])
            nc.vector.tensor_copy(st[:, 2, :], psum_h[:])
            # last block's store goes on the (by-then idle) scalar HWDGE
            # ring: ~1us lower first-byte latency than SWDGE shortens the tail
            (nc.scalar if bi == BPC - 1 else nc.gpsimd).dma_start(out_d[bi], st[:])
            if bi < BPC - 1:
                # a short same-stationary burst after each block keeps the
                # HAM activity monitor reading "busy" (real matmuls reload
                # the stationary every time, which HAM seems to discount);
                # kept small so the PE never becomes the critical path.
                # Extra cushion near the tail, where arrival jitter is worst.
                # Full-width (N=448) dummies are required: half-width ones
                # measured 92us — the activity monitor counts busy wall-time
                # coverage, so cheaper bursts let the clock re-throttle.
                for _ in range(NDUMMY):
                    nc.tensor.matmul(pw[:], wt[:, :128], wt[:],
                                     start=True, stop=True)
    nc.finalize()
    return nc


def prepare_inputs(x, w1, w2, w3, pad_hv, idx_identit):
    """Host-side shard prep. Returns in_maps (list of 8 dicts)."""
    x = np.asarray(x)
    xb = x.astype(BF16)                                   # (B, C, 60, 60)
    # h-major for V/I: [c, h', (b, w in [2,58))]
    x_hbw = np.ascontiguousarray(
        xb[:, :, :, EP:EP + WOUT].transpose(1, 2, 0, 3)).reshape(C_IN * HIN, NFREE)
    # w-major for H: [c, w', (b, h in [2,58))]
    x_wbh = np.ascontiguousarray(
        xb[:, :, EP:EP + HOUT, :].transpose(1, 3, 0, 2)).reshape(C_IN * WIN, NFREE)

    opv, oph = _build_operators(w1, w2, w3, pad_hv, idx_identit)
    opv16 = opv.astype(BF16)                               # (96, 660, 120)
    oph16 = oph.astype(BF16)                               # (96, 660, 56)

    nm = NCH * KCH                                         # 640 main rows

    def chunked(a):  # (BPC, 640, M) -> (BPC, KCH, NCH, M)
        m = a.shape[2]
        return a.reshape(BPC, NCH, KCH, m).transpose(0, 2, 1, 3)

    in_maps = []
    for i in range(N_CORES):
        r0 = i * CPC * HIN
        xv_r = x_hbw[r0:r0 + CPC * HIN].reshape(BPC, KROWS, NFREE)
        xh_r = x_wbh[r0:r0 + CPC * WIN].reshape(BPC, KROWS, NFREE)
        ov = opv16[i * BPC:(i + 1) * BPC]                  # (BPC, 660, 120)
        oh = oph16[i * BPC:(i + 1) * BPC]
        xall = np.concatenate(
            [chunked(xv_r[:, :nm]), chunked(xh_r[:, :nm]),
             chunked(ov[:, :nm]), chunked(oh[:, :nm])],
            axis=3)                                        # (BPC, 128, 5, 1072)
        xtail = np.concatenate(
            [xv_r[:, nm:], xh_r[:, nm:], ov[:, nm:], oh[:, nm:]],
            axis=2)                                        # (BPC, 20, 1072)
        in_maps.append({
            "xall": np.ascontiguousarray(xall),
            "xtail": np.ascontiguousarray(xtail.transpose(1, 0, 2)),
        })
    return in_maps


def unshard(results):
    """results: list of 8 dicts with 'out' (BPC, 56, 3, 448) bf16 ->
    (out_h, out_v, out_i) each (B, C_OUT, 56, 56) fp32."""
    O = np.stack([np.asarray(r["out"], np.float32) for r in results])  # (8,12,56,3,448)
    O = O.reshape(N_CORES, BPC, 56, 3, B, WOUT)
    # (core, co_l, h, b, w) -> (b, core, co_l, h, w)
    out_v = O[:, :, :, 0].transpose(3, 0, 1, 2, 4).reshape(B, C_OUT, HOUT, WOUT)
    out_i = O[:, :, :, 1].transpose(3, 0, 1, 2, 4).reshape(B, C_OUT, HOUT, WOUT)
    h = O[:, :, :, 2]                          # (core, co_l, w, b, h)
    out_h = h.transpose(3, 0, 1, 4, 2).reshape(B, C_OUT, HOUT, WOUT)
    return out_h, out_v, out_i


def kernel(x, w1, w2, w3, pad_hv, idx_identit, b=B, hout=HOUT, wout=WOUT):
    from concourse.bass_utils import run_bass_kernel_spmd

    assert int(b) == B and int(hout) == HOUT and int(wout) == WOUT
    assert tuple(np.asarray(x).shape) == (B, C_IN, HIN, WIN)

    in_maps = prepare_inputs(x, w1, w2, w3, pad_hv, idx_identit)
    nc = _CACHE.get("nc")
    if nc is None:
        nc = _build_nc()
        _CACHE["nc"] = nc
    res = run_bass_kernel_spmd(nc, in_maps, core_ids=list(range(N_CORES)))
    return unshard(res.results)
